# revision 1
# baseline (speedup 1.0000x reference)
"""DC_CE_Marginal_loss for Trainium2 — 8-core data-parallel Bass kernel.

Shards the [B,C,D,H,W] volume along D across 8 NeuronCores, two launches:

  Launch A: each core loads its target shard (bf16; one-hot is exact) and
      computes local per-(b,c) voxel counts (free-dim reductions split over
      ScalarE and VectorE). Host sums the 8x[128,16] partials into global
      counts — the "psum of present-class counts" — and derives the
      presence masks / merge weights / CE padding (40 floats).

  Launch B: each core streams its net_output shard and computes per chunk:
      merged background logit (masked scalar_tensor_tensor chain), masked
      exp (ACT, additive -1e9 bias), softmax denominator S (pairwise adds),
      fast reciprocal, then fused affine_mul_reduce ops that produce
      softmax q while accumulating seg_vol / intersect / sum(t*m) into
      per-chunk columns; ACT Log accumulates sum(log(S+pad)).

Host sums the per-core/per-chunk partial columns and finishes the loss.
"""
import numpy as np
import ml_dtypes

B, C, D, H, W = 2, 8, 64, 160, 160
NCORES = 8
DS = D // NCORES            # depth slices per core
PLANE = DS * H * W          # voxels per (b,c) plane per core = 204800
P = 128
FREE = PLANE // P           # 1600
NCH = 4                     # chunks per sample plane
FCH = FREE // NCH           # 400
BIG = 1e9
NVOX = B * D * H * W

# launch B per-chunk accumulator columns: base = (b*NCH+ch)*CPC
CPC = 25          # seg[0:8], intersect[8:16], u-terms[16:24], lse[24]
NOUT = B * NCH * CPC
# masks input columns
MK_BM = 0         # 16: additive exp mask (0 present / -BIG absent)
MK_A = 16         # 16: 1-present (merge weights)
MK_PAD = 32       # 2: CE padding per sample
NMASK = 40

_CACHE = {}


def _build_a():
    import concourse.bacc as bacc
    import concourse.tile as tile
    from concourse import mybir

    FA = mybir.ActivationFunctionType
    AL = mybir.AluOpType
    f32, bf16 = mybir.dt.float32, mybir.dt.bfloat16

    nc = bacc.Bacc("TRN2", num_devices=NCORES, name="loss_counts")
    t = nc.dram_tensor("t", [B * C, P, FREE], bf16, kind="ExternalInput")
    out = nc.dram_tensor("cnt", [P, B * C], f32, kind="ExternalOutput")

    with tile.TileContext(nc) as tc:
        with (
            tc.tile_pool(name="tin", bufs=4) as tin,
            tc.tile_pool(name="sb", bufs=1) as sb,
        ):
            cnt = sb.tile([P, B * C], f32)
            junk_a = sb.tile([P, 2 * FREE], bf16)
            for g in range(8):  # 2 planes per DMA; reduce on DVE or ACT
                t_sb = tin.tile([P, 2, FREE], bf16, tag="t")
                src = t[2 * g : 2 * g + 2, :, :].rearrange("c p f -> p c f")
                nc.sync.dma_start(t_sb[:], src)
                if g % 2 == 0:
                    nc.vector.tensor_reduce(
                        out=cnt[:, 2 * g : 2 * g + 2], in_=t_sb[:],
                        axis=mybir.AxisListType.X, op=AL.add)
                else:
                    for j in range(2):
                        nc.scalar.activation(
                            out=junk_a[:, j * FREE : (j + 1) * FREE],
                            in_=t_sb[:, j, :], func=FA.Copy,
                            accum_out=cnt[:, 2 * g + j : 2 * g + j + 1])
            nc.sync.dma_start(out[:], cnt[:])
    nc.compile()
    return nc


import os
BG_GPSIMD = os.environ.get("K_BG_GPSIMD", "0") == "1"
E_BF16 = os.environ.get("K_E_BF16", "1") == "1"
I_ON_ACT = os.environ.get("K_I_ON_ACT", "1") == "1"
UM_GPSIMD = os.environ.get("K_UM_GPSIMD", "0") == "1"
UM_WIDE_TT = os.environ.get("K_UM_WIDE_TT", "1") == "1"


def _build_b():
    import concourse.bacc as bacc
    import concourse.tile as tile
    from concourse import mybir

    FA = mybir.ActivationFunctionType
    AL = mybir.AluOpType
    f32, bf16 = mybir.dt.float32, mybir.dt.bfloat16
    edt = bf16 if E_BF16 else f32

    nc = bacc.Bacc("TRN2", num_devices=NCORES, name="loss_main")
    x = nc.dram_tensor("x", [B * C, P, FREE], f32, kind="ExternalInput")
    t = nc.dram_tensor("t", [B * C, P, FREE], bf16, kind="ExternalInput")
    masks = nc.dram_tensor("masks", [P, NMASK], f32, kind="ExternalInput")
    out = nc.dram_tensor("out", [P, NOUT], f32, kind="ExternalOutput")

    beng = nc.gpsimd if BG_GPSIMD else nc.vector

    with tile.TileContext(nc) as tc:
        with (
            tc.tile_pool(name="persist", bufs=1) as persist,
            tc.tile_pool(name="xin", bufs=3) as xin,
            tc.tile_pool(name="ework", bufs=2) as ework,
            tc.tile_pool(name="qwork", bufs=2) as qwork,
            tc.tile_pool(name="swork", bufs=2) as swork,
        ):
            mk = persist.tile([P, NMASK], f32)
            nc.sync.dma_start(mk[:], masks[:])
            # prefetch chunk-0 logits before the (large) target loads so the
            # first chunk's DVE work isn't gated on all 6.6MB of t
            x_ch0 = xin.tile([P, C, FCH], f32, tag="x", name="x_ch0")
            nc.sync.dma_start(
                x_ch0[:], x[0:C, :, 0:FCH].rearrange("c p f -> p c f"))
            t_sb = persist.tile([P, B * C, FREE], bf16)
            for bc in range(B * C):
                nc.sync.dma_start(t_sb[:, bc, :], t[bc])
            accs = persist.tile([P, NOUT], f32)
            nc.vector.memset(accs[:], 0.0)
            junk_dve = persist.tile([P, C, FCH], f32)
            # all S chunks retained so the Ln ops run back-to-back at the
            # end (one act-table load instead of per-chunk exp<->ln flips)
            S_all = persist.tile([P, B * NCH, FCH], f32)

            for b in range(B):
                for ch in range(NCH):
                    sl = slice(ch * FCH, (ch + 1) * FCH)
                    base = (b * NCH + ch) * CPC
                    if b == 0 and ch == 0:
                        x_ch = x_ch0
                    else:
                        x_ch = xin.tile([P, C, FCH], f32, tag="x")
                        src = x[b * C : (b + 1) * C, :, sl].rearrange(
                            "c p f -> p c f")
                        nc.sync.dma_start(x_ch[:], src)

                    # bg = sum_{c>=1} absent_c * x_c ; x_0 += bg (merged logit)
                    bg = swork.tile([P, FCH], f32, tag="bg")
                    beng.tensor_scalar(
                        bg[:], x_ch[:, 1, :],
                        mk[:, MK_A + b * C + 1 : MK_A + b * C + 2], None, AL.mult)
                    for c in range(2, C):
                        bg2 = swork.tile([P, FCH], f32, tag="bg")
                        beng.scalar_tensor_tensor(
                            out=bg2[:], in0=x_ch[:, c, :],
                            scalar=mk[:, MK_A + b * C + c : MK_A + b * C + c + 1],
                            in1=bg[:], op0=AL.mult, op1=AL.add)
                        bg = bg2
                    beng.scalar_tensor_tensor(
                        out=x_ch[:, 0, :], in0=x_ch[:, 0, :], scalar=1.0,
                        in1=bg[:], op0=AL.mult, op1=AL.add)

                    # e_c = exp(m_c + mask_bias_c)
                    e_ch = ework.tile([P, C, FCH], edt, tag="e")
                    for c in range(C):
                        last_exp = nc.scalar.activation(
                            out=e_ch[:, c, :], in_=x_ch[:, c, :],
                            func=FA.Exp,
                            bias=mk[:, MK_BM + b * C + c : MK_BM + b * C + c + 1],
                            scale=1.0)

                    # S = sum_c e_c (pairwise tree on wide slices)
                    s4 = swork.tile([P, 4, FCH], edt, tag="s4")
                    nc.vector.tensor_tensor(out=s4[:], in0=e_ch[:, 0:4, :],
                                            in1=e_ch[:, 4:8, :], op=AL.add)
                    s2 = swork.tile([P, 2, FCH], edt, tag="s2")
                    nc.vector.tensor_tensor(out=s2[:], in0=s4[:, 0:2, :],
                                            in1=s4[:, 2:4, :], op=AL.add)
                    S = S_all[:, b * NCH + ch, :]
                    nc.vector.tensor_tensor(out=S, in0=s2[:, 0, :],
                                            in1=s2[:, 1, :], op=AL.add)

                    r = swork.tile([P, FCH], f32, tag="r")
                    nc.vector.reciprocal_approx_fast(r[:], S)

                    # q_c = e_c * r ; seg_c = sum(q_c)  (fused custom DVE op)
                    q_ch = qwork.tile([P, C, FCH], edt, tag="q")
                    for c in range(C):
                        nc.vector.affine_mul_reduce(
                            out=q_ch[:, c, :],
                            accum_out=accs[:, base + c : base + c + 1],
                            in0=e_ch[:, c, :], in1=r[:], scale=1.0, bias=0.0)
                    # intersect_c = sum(t_c * q_c)
                    if I_ON_ACT:
                        tq_ch = qwork.tile([P, C, FCH], edt, tag="tq")
                        nc.vector.tensor_tensor(
                            out=tq_ch[:], in0=t_sb[:, b * C : (b + 1) * C, sl],
                            in1=q_ch[:], op=AL.mult)
                        for c in range(C):
                            nc.scalar.activation(
                                out=tq_ch[:, c, :], in_=tq_ch[:, c, :],
                                func=FA.Copy,
                                accum_out=accs[:, base + 8 + c : base + 9 + c])
                    else:
                        for c in range(C):
                            nc.vector.affine_mul_reduce(
                                out=junk_dve[:, 0, :],
                                accum_out=accs[:, base + 8 + c : base + 9 + c],
                                in0=t_sb[:, b * C + c, sl], in1=q_ch[:, c, :],
                                scale=1.0, bias=0.0)
                    # u-term = sum_c sum(t_c * m_c)   (x_0 already merged)
                    if UM_WIDE_TT:
                        um_ch = qwork.tile([P, C, FCH], f32, tag="um")
                        ueng = nc.gpsimd if UM_GPSIMD else nc.vector
                        ueng.tensor_tensor(
                            out=um_ch[:], in0=t_sb[:, b * C : (b + 1) * C, sl],
                            in1=x_ch[:, :, :], op=AL.mult)
                        nc.scalar.activation(
                            out=um_ch[:], in_=um_ch[:], func=FA.Copy,
                            accum_out=accs[:, base + 16 : base + 17])
                    else:
                        for c in range(C):
                            nc.vector.affine_mul_reduce(
                                out=junk_dve[:, 0, :],
                                accum_out=accs[:, base + 16 + c : base + 17 + c],
                                in0=t_sb[:, b * C + c, sl],
                                in1=x_ch[:, c, :],
                                scale=1.0, bias=0.0)

            # CE lse terms at the end: sum(log(S + pad_b)) via ACT accum
            junk_act = persist.tile([P, FCH], f32)
            from concourse.tile import add_dep_helper
            for b in range(B):
                for ch in range(NCH):
                    base = (b * NCH + ch) * CPC
                    ln_inst = nc.scalar.activation(
                        out=junk_act[:], in_=S_all[:, b * NCH + ch, :],
                        func=FA.Ln,
                        bias=mk[:, MK_PAD + b : MK_PAD + b + 1], scale=1.0,
                        accum_out=accs[:, base + 24 : base + 25])
                    # keep every Ln after the final Exp so the activation
                    # table set is switched exactly once
                    add_dep_helper(ln_inst.ins, last_exp.ins, False,
                                   "batch ln after exps")

            nc.sync.dma_start(out[:], accs[:])
    nc.compile()
    return nc


def _get(name, builder):
    if name not in _CACHE:
        _CACHE[name] = builder()
    return _CACHE[name]


def _shard_inputs(net_output, target):
    xs = np.ascontiguousarray(net_output).reshape(B, C, NCORES, P, FREE)
    ts = np.ascontiguousarray(target).reshape(B, C, NCORES, P, FREE)
    xmaps, tmaps = [], []
    for k in range(NCORES):
        xk = np.ascontiguousarray(xs[:, :, k]).reshape(B * C, P, FREE)
        tk = np.ascontiguousarray(ts[:, :, k]).reshape(B * C, P, FREE)
        xmaps.append(xk)
        tmaps.append(tk.astype(ml_dtypes.bfloat16))  # one-hot: exact in bf16
    return xmaps, tmaps


def _masks_from_counts(cnt_g):
    """cnt_g [B,C] -> (masks [P,NMASK] f32, present, n)"""
    present = cnt_g > 0.5
    pm = present.astype(np.float32)
    n = pm.sum(axis=1)
    L = n.max()
    pad = (L - n).astype(np.float32)
    mrow = np.zeros((NMASK,), dtype=np.float32)
    mrow[MK_BM : MK_BM + B * C] = pm.reshape(-1) * BIG - BIG
    mrow[MK_A : MK_A + B * C] = 1.0 - pm.reshape(-1)
    mrow[MK_PAD : MK_PAD + B] = pad
    masks = np.ascontiguousarray(np.broadcast_to(mrow, (P, NMASK)))
    return masks, present, n


def _run(nc, in_maps, out_name):
    if os.environ.get("K_SIM", "0") == "1":
        import concourse.bass_interp as bass_interp
        sim = bass_interp.MultiCoreSim(nc, NCORES)
        for k in range(NCORES):
            for name, arr in in_maps[k].items():
                sim.cores[k].tensor(name)[:] = arr
        sim.simulate()
        return [{out_name: sim.cores[k].tensor(out_name).copy()}
                for k in range(NCORES)]
    from concourse.bass_utils import run_bass_kernel_spmd
    return run_bass_kernel_spmd(
        nc, in_maps, core_ids=list(range(NCORES))).results


def run_a(tmaps):
    nc = _get("a", _build_a)
    results = _run(nc, [{"t": tk} for tk in tmaps], "cnt")
    cnt_g = np.zeros((B, C), dtype=np.float64)
    for r in results:
        cnt_g += r["cnt"].astype(np.float64).sum(axis=0).reshape(B, C)
    return cnt_g


def run_b(xmaps, tmaps, masks):
    nc = _get("b", _build_b)
    in_maps = [{"x": xmaps[k], "t": tmaps[k], "masks": masks}
               for k in range(NCORES)]
    results = _run(nc, in_maps, "out")
    acc = np.zeros((NOUT,), dtype=np.float64)
    for r in results:
        acc += r["out"].astype(np.float64).sum(axis=0)
    return acc


def _finish(cnt_g, acc, present, n):
    cols = acc.reshape(B, NCH, CPC).sum(axis=1)   # [B, CPC]
    seg = cols[:, 0:8]
    inter = cols[:, 8:16]
    u = cols[:, 16:24].sum(axis=1)                # [B]
    lse_sum = cols[:, 24]
    ce = (lse_sum.sum() - u.sum()) / NVOX
    dice_c = 2.0 * inter / (cnt_g + seg + 1e-5)
    dice_i = 1.0 - (present * dice_c).sum(axis=1) / n
    dc = dice_i.mean()
    return np.asarray(0.5 * ce + 0.5 * dc, dtype=np.float32)


def kernel(net_output, target):
    xmaps, tmaps = _shard_inputs(np.asarray(net_output), np.asarray(target))
    cnt_g = run_a(tmaps)
    masks, present, n = _masks_from_counts(cnt_g)
    acc = run_b(xmaps, tmaps, masks)
    return _finish(cnt_g, acc, present, n)



# revision 14
# speedup vs baseline: 1.6757x; 1.6757x over previous
"""DC_CE_Marginal_loss for Trainium2 — 8-core data-parallel Bass kernel (v2).

Shards the [B,C,D,H,W] volume along D across 8 NeuronCores, two launches:

  Launch A (counts): per-core fp8 one-hot target (exact in fp8-e4m3) is
      streamed once; per-(b,c) voxel counts come from free-dim reductions
      split across DVE / ACT. Host psums the 8x[128,16] partials into
      global counts and derives the per-sample present-class pattern +
      CE padding.

  Launch B (main, compiled per present-pattern): all-bf16 pipeline.
      Per sample: bg-merge only over the absent channels (weights are
      exactly 1), one wide exp over the present channels (present bias
      is 0 so no mask add is needed), pairwise-tree softmax denominator,
      fast reciprocal, then per-channel tensor_tensor_reduce ops that
      produce q = e*r and tq = t*q while accumulating seg_vol / intersect
      into per-(b,c) columns. q/tq overwrite the dead x/t planes in SBUF.
      The CE dot product sum(t*m) is recovered analytically:
      m_label = ln(q_label) + ln(S), and q_label = sum_c tq_c (an exact
      one-hot select), so two ACT Ln+accum passes replace the whole t*m
      multiply/accumulate pipeline.

Host sums the per-core partial columns and finishes the loss.
"""
import numpy as np
import ml_dtypes

B, C, D, H, W = 2, 8, 64, 160, 160
NCORES = 8
DS = D // NCORES            # depth slices per core
PLANE = DS * H * W          # voxels per (b,c) plane per core = 204800
P = 128
FREE = PLANE // P           # 1600
NVOX = B * D * H * W

# launch B accumulator columns.
# SEGB/INTB columns hold per-(b,c) values at PARTITION c (from the matmul
# reduction), so the host reads them per-partition instead of summing.
SEGB = 0           # B cols: seg_vol, value for channel c at partition c
INTB = 2           # B cols: intersect, value for channel c at partition c
U1 = 4             # 2: sum ln(q_label) per sample (summed over partitions)
U2 = 6             # 2: sum ln(S) per sample (only used when pad>0)
LSE = 8            # 2: sum ln(S + pad) per sample
NACC = 10

_CACHE = {}


def _build_a():
    import concourse.bacc as bacc
    import concourse.tile as tile
    from concourse import mybir

    FA = mybir.ActivationFunctionType
    AL = mybir.AluOpType
    f32 = mybir.dt.float32
    f8 = mybir.dt.float8e4

    nc = bacc.Bacc("TRN2", num_devices=NCORES, name="loss_counts_v2")
    t = nc.dram_tensor("t", [B * C, P, FREE], f8, kind="ExternalInput")
    out = nc.dram_tensor("cnt", [P, B * C], f32, kind="ExternalOutput")

    with tile.TileContext(nc) as tc:
        with (
            tc.tile_pool(name="tin", bufs=8) as tin,
            tc.tile_pool(name="sb", bufs=1) as sb,
        ):
            cnt = sb.tile([P, B * C], f32)
            junk_a = sb.tile([P, FREE], f32)
            # 8 groups of 2 planes; alternate DVE reduce / ACT accum
            for g in range(8):
                t_sb = tin.tile([P, 2, FREE], f8, tag="t")
                src = t[2 * g : 2 * g + 2, :, :].rearrange("c p f -> p c f")
                nc.sync.dma_start(t_sb[:], src)
                if g % 2 == 0:
                    nc.vector.tensor_reduce(
                        out=cnt[:, 2 * g : 2 * g + 2], in_=t_sb[:],
                        axis=mybir.AxisListType.X, op=AL.add)
                else:
                    for j in range(2):
                        nc.scalar.activation(
                            out=junk_a[:], in_=t_sb[:, j, :], func=FA.Copy,
                            accum_out=cnt[:, 2 * g + j : 2 * g + j + 1])
            nc.sync.dma_start(out[:], cnt[:])
    nc.compile()
    return nc


def _tree(nc, s4, s2, planes, out_ap=None):
    """Pairwise-add a list of [P, F] APs using slices of the scratch tiles
    s4 ([P,4,F]) / s2 ([P,2,F]) for intermediates. The final add writes
    out_ap if given. Returns the final AP. len(planes) in [2, 8]."""
    from concourse import mybir
    AL = mybir.AluOpType
    scratch = [s4, s2, None]
    cur = list(planes)
    li = 0
    while len(cur) > 1:
        nxt = []
        k = 0
        for i in range(0, len(cur) - 1, 2):
            final = len(cur) == 2
            if final and out_ap is not None:
                dst = out_ap
            elif final and out_ap is None:
                # pick a slot that is never an input at this level
                dst = (s2 if li < 2 else s4)[:, 0, :]
            else:
                dst = scratch[li][:, k, :]
                k += 1
            nc.vector.tensor_tensor(out=dst, in0=cur[i], in1=cur[i + 1],
                                    op=AL.add)
            nxt.append(dst)
        if len(cur) % 2:
            nxt.append(cur[-1])
        cur = nxt
        li += 1
    return cur[0]


def _build_b(pattern):
    """pattern: tuple per sample of present-channel tuples."""
    import concourse.bacc as bacc
    import concourse.tile as tile
    from concourse import mybir

    FA = mybir.ActivationFunctionType
    AL = mybir.AluOpType
    f32, bf16 = mybir.dt.float32, mybir.dt.bfloat16

    pres = [list(p) for p in pattern]
    n = [len(p) for p in pres]
    L = max(n)
    pad = [float(L - nn) for nn in n]
    absent = [[c for c in range(C) if c not in p] for p in pres]

    nc = bacc.Bacc("TRN2", num_devices=NCORES, name="loss_main_v2")
    x = nc.dram_tensor("x", [B * C, P, FREE], bf16, kind="ExternalInput")
    t = nc.dram_tensor("t", [B * C, P, FREE], bf16, kind="ExternalInput")
    out = nc.dram_tensor("out", [P, NACC], f32, kind="ExternalOutput")

    with tile.TileContext(nc) as tc:
        with (
            tc.tile_pool(name="persist", bufs=1) as persist,
            tc.tile_pool(name="ework", bufs=2) as ework,
            tc.tile_pool(name="small", bufs=2) as small,
            tc.psum_pool(name="ps", bufs=2) as psp,
        ):
            x_sb = persist.tile([P, B * C, FREE], bf16)
            # only present t planes are ever read — pack them
            tslot = {}
            for b in range(B):
                for c in pres[b]:
                    tslot[(b, c)] = len(tslot)
            t_sb = persist.tile([P, len(tslot), FREE], bf16)
            accs = persist.tile([P, NACC], f32)
            s4 = persist.tile([P, 4, FREE], bf16)
            s2 = persist.tile([P, 2, FREE], bf16)
            junk = persist.tile([P, FREE], bf16)
            nc.vector.memset(accs[:], 0.0)
            # sliding-window ones for per-channel matmul reductions:
            # W_c = wsl[:, C-1-c : 2*C-1-c] is [128, C] with ones in col c
            wsl = persist.tile([P, 2 * C - 1], bf16)
            nc.gpsimd.memset(wsl[:], 0.0)
            nc.gpsimd.memset(wsl[:, C - 1 : C], 1.0)
            CH = [(0, 400), (400, 800), (800, 1200), (1200, 1600)]

            def class_sums(planes, acc_col):
                """acc_col[c] (partition c) = sum over plane c, via PE."""
                ps = psp.tile([C, 400], f32, tag="ps")
                items = [(c, j) for c in range(len(planes))
                         for j in range(len(CH))]
                for idx, (c, j) in enumerate(items):
                    lo, hi = CH[j]
                    nc.tensor.matmul(
                        ps[:, 0 : hi - lo],
                        wsl[:, C - 1 - c : 2 * C - 1 - c],
                        planes[c][:, lo:hi],
                        start=(idx == 0), stop=(idx == len(items) - 1))
                nc.vector.tensor_reduce(
                    out=acc_col, in_=ps[:],
                    axis=mybir.AxisListType.X, op=AL.add)

            # DMA: per sample, absent x planes first (bg tree), then
            # present; then the sample's present t planes.
            for b in range(B):
                for c in absent[b] + pres[b]:
                    bc = b * C + c
                    nc.sync.dma_start(x_sb[:, bc, :], x[bc])
            for b in range(B):
                for c in pres[b]:
                    nc.sync.dma_start(t_sb[:, tslot[(b, c)], :], t[b * C + c])

            for b in range(B):
                xb = x_sb[:, b * C : (b + 1) * C, :]
                tb = lambda c: t_sb[:, tslot[(b, c)], :]
                # ---- bg merge: x0 += sum(absent x) (weights exactly 1)
                if len(absent[b]) == 1:
                    nc.vector.tensor_tensor(
                        out=xb[:, 0, :], in0=xb[:, 0, :],
                        in1=xb[:, absent[b][0], :], op=AL.add)
                elif absent[b]:
                    bg = _tree(nc, s4, s2, [xb[:, c, :] for c in absent[b]])
                    nc.vector.tensor_tensor(
                        out=xb[:, 0, :], in0=xb[:, 0, :], in1=bg, op=AL.add)

                # ---- e = exp(x) over present channels (contiguous runs)
                e = ework.tile([P, C, FREE], bf16, tag="e")
                runs = []
                for c in pres[b]:
                    if runs and runs[-1][1] == c:
                        runs[-1][1] = c + 1
                    else:
                        runs.append([c, c + 1])
                for lo, hi in runs:
                    nc.scalar.activation(
                        out=e[:, lo:hi, :], in_=xb[:, lo:hi, :], func=FA.Exp)

                # ---- S = sum_present e (f32); r = 1/S; rb = bf16 r
                S = small.tile([P, FREE], f32, tag="S")
                _tree(nc, s4, s2, [e[:, c, :] for c in pres[b]], out_ap=S[:])
                r = small.tile([P, FREE], f32, tag="r")
                nc.vector.reciprocal_approx_fast(r[:], S[:])
                rb = small.tile([P, FREE], bf16, tag="rb")
                nc.vector.tensor_scalar(rb[:], r[:], 1.0, None, AL.mult)

                # ---- q_c = e_c * r, overwriting x planes;
                #      tq_c = t_c * q_c, overwriting t planes;
                #      seg/intersect reduced per class on the PE
                for c in pres[b]:
                    nc.vector.tensor_tensor(
                        out=xb[:, c, :], in0=e[:, c, :], in1=rb[:],
                        op=AL.mult)
                class_sums([xb[:, c, :] for c in pres[b]],
                           accs[0:C, SEGB + b : SEGB + b + 1])
                for c in pres[b]:
                    nc.vector.tensor_tensor(
                        out=tb(c), in0=tb(c), in1=xb[:, c, :],
                        op=AL.mult)
                class_sums([tb(c) for c in pres[b]],
                           accs[0:C, INTB + b : INTB + b + 1])

                # ---- g_q = sum_c tq_c = q_label (exact one-hot select)
                gq = small.tile([P, FREE], bf16, tag="gq")
                _tree(nc, s4, s2, [tb(c) for c in pres[b]],
                      out_ap=gq[:])

                # ---- CE terms: u = sum ln(g_q) + sum ln(S); lse = ln(S+pad)
                nc.scalar.activation(
                    out=junk[:], in_=gq[:], func=FA.Ln,
                    accum_out=accs[:, U1 + b : U1 + b + 1])
                if pad[b] > 0:
                    padb = small.tile([P, 1], f32, tag="pad")
                    nc.vector.memset(padb[:], pad[b])
                    nc.scalar.activation(
                        out=junk[:], in_=S[:], func=FA.Ln, bias=padb[:],
                        accum_out=accs[:, LSE + b : LSE + b + 1])
                    nc.scalar.activation(
                        out=junk[:], in_=S[:], func=FA.Ln,
                        accum_out=accs[:, U2 + b : U2 + b + 1])
                else:
                    nc.scalar.activation(
                        out=junk[:], in_=S[:], func=FA.Ln,
                        accum_out=accs[:, LSE + b : LSE + b + 1])

            nc.sync.dma_start(out[:], accs[:])
    nc.compile()
    return nc


def _get(name, builder, *args):
    if name not in _CACHE:
        _CACHE[name] = builder(*args)
    return _CACHE[name]


def _shard_inputs(net_output, target):
    xs = np.ascontiguousarray(net_output).reshape(B, C, NCORES, P, FREE)
    ts = np.ascontiguousarray(target).reshape(B, C, NCORES, P, FREE)
    xmaps, tmaps, t8maps = [], [], []
    for k in range(NCORES):
        xk = np.ascontiguousarray(xs[:, :, k]).reshape(B * C, P, FREE)
        tk = np.ascontiguousarray(ts[:, :, k]).reshape(B * C, P, FREE)
        xmaps.append(xk.astype(ml_dtypes.bfloat16))
        tmaps.append(tk.astype(ml_dtypes.bfloat16))   # one-hot: exact
        t8maps.append(tk.astype(ml_dtypes.float8_e4m3))  # exact in fp8
    return xmaps, tmaps, t8maps


def _run(nc, in_maps, out_name):
    import os
    if os.environ.get("K_SIM", "0") == "1":
        import concourse.bass_interp as bass_interp
        sim = bass_interp.MultiCoreSim(nc, NCORES)
        for k in range(NCORES):
            for name, arr in in_maps[k].items():
                sim.cores[k].tensor(name)[:] = arr
        sim.simulate()
        return [{out_name: sim.cores[k].tensor(out_name).copy()}
                for k in range(NCORES)]
    from concourse.bass_utils import run_bass_kernel_spmd
    return run_bass_kernel_spmd(
        nc, in_maps, core_ids=list(range(NCORES))).results


def run_a(t8maps):
    nc = _get("a", _build_a)
    results = _run(nc, [{"t": tk} for tk in t8maps], "cnt")
    cnt_g = np.zeros((B, C), dtype=np.float64)
    for r in results:
        cnt_g += r["cnt"].astype(np.float64).sum(axis=0).reshape(B, C)
    return cnt_g


def run_b(xmaps, tmaps, pattern):
    nc = _get(("b", pattern), _build_b, pattern)
    in_maps = [{"x": xmaps[k], "t": tmaps[k]} for k in range(NCORES)]
    results = _run(nc, in_maps, "out")
    acc = np.zeros((P, NACC), dtype=np.float64)
    for r in results:
        acc += r["out"].astype(np.float64)
    return acc


def _finish(cnt_g, acc, present, n):
    pad = n.max() - n
    # SEGB/INTB: per-class values live at partition = position in the
    # present-channel list of that sample
    seg = np.zeros((B, C)); inter = np.zeros((B, C))
    for b in range(B):
        pres = np.where(present[b])[0]
        seg[b, pres] = acc[: len(pres), SEGB + b]
        inter[b, pres] = acc[: len(pres), INTB + b]
    cols = acc.sum(axis=0)
    u1 = cols[U1 : U1 + B]
    lse = cols[LSE : LSE + B]
    u2 = np.where(pad > 0, cols[U2 : U2 + B], lse)
    ce = (lse.sum() - (u1.sum() + u2.sum())) / NVOX
    dice_c = 2.0 * inter / (cnt_g + seg + 1e-5)
    dice_i = 1.0 - (present * dice_c).sum(axis=1) / n
    dc = dice_i.mean()
    return np.asarray(0.5 * ce + 0.5 * dc, dtype=np.float32)


def kernel(net_output, target):
    xmaps, tmaps, t8maps = _shard_inputs(
        np.asarray(net_output), np.asarray(target))
    cnt_g = run_a(t8maps)
    present = cnt_g > 0.5
    n = present.sum(axis=1).astype(np.float64)
    pattern = tuple(tuple(int(c) for c in np.where(present[b])[0])
                    for b in range(B))
    acc = run_b(xmaps, tmaps, pattern)
    return _finish(cnt_g, acc, present, n)


# revision 15
# speedup vs baseline: 1.7069x; 1.0186x over previous
"""DC_CE_Marginal_loss for Trainium2 — 8-core data-parallel Bass kernel (v2).

Shards the [B,C,D,H,W] volume along D across 8 NeuronCores, two launches:

  Launch A (counts): per-core fp8 one-hot target (exact in fp8-e4m3) is
      streamed once; per-(b,c) voxel counts come from free-dim reductions
      split across DVE / ACT. Host psums the 8x[128,16] partials into
      global counts and derives the per-sample present-class pattern +
      CE padding.

  Launch B (main, compiled per present-pattern): all-bf16 pipeline.
      Per sample: bg-merge only over the absent channels (weights are
      exactly 1), one wide exp over the present channels (present bias
      is 0 so no mask add is needed), pairwise-tree softmax denominator,
      fast reciprocal, then per-channel tensor_tensor_reduce ops that
      produce q = e*r and tq = t*q while accumulating seg_vol / intersect
      into per-(b,c) columns. q/tq overwrite the dead x/t planes in SBUF.
      The CE dot product sum(t*m) is recovered analytically:
      m_label = ln(q_label) + ln(S), and q_label = sum_c tq_c (an exact
      one-hot select), so two ACT Ln+accum passes replace the whole t*m
      multiply/accumulate pipeline.

Host sums the per-core partial columns and finishes the loss.
"""
import numpy as np
import ml_dtypes

B, C, D, H, W = 2, 8, 64, 160, 160
NCORES = 8
DS = D // NCORES            # depth slices per core
PLANE = DS * H * W          # voxels per (b,c) plane per core = 204800
P = 128
FREE = PLANE // P           # 1600
NVOX = B * D * H * W

# launch B accumulator columns.
# SEGB/INTB columns hold per-(b,c) values at PARTITION c (from the matmul
# reduction), so the host reads them per-partition instead of summing.
SEGB = 0           # B cols: seg_vol, value for channel c at partition c
INTB = 2           # B cols: intersect, value for channel c at partition c
U1 = 4             # 2: sum ln(q_label) per sample (summed over partitions)
U2 = 6             # 2: sum ln(S) per sample (only used when pad>0)
LSE = 8            # 2: sum ln(S + pad) per sample
NACC = 10

_CACHE = {}


def _build_a():
    import concourse.bacc as bacc
    import concourse.tile as tile
    from concourse import mybir

    FA = mybir.ActivationFunctionType
    AL = mybir.AluOpType
    f32 = mybir.dt.float32
    f8 = mybir.dt.float8e4

    nc = bacc.Bacc("TRN2", num_devices=NCORES, name="loss_counts_v2")
    # partition-major: per partition the B*C*FREE block is contiguous, so
    # one DMA covers many planes with large descriptors
    t = nc.dram_tensor("t", [P, B * C, FREE], f8, kind="ExternalInput")
    out = nc.dram_tensor("cnt", [P, B * C], f32, kind="ExternalOutput")

    with tile.TileContext(nc) as tc:
        with (
            tc.tile_pool(name="tin", bufs=4) as tin,
            tc.tile_pool(name="sb", bufs=1) as sb,
        ):
            cnt = sb.tile([P, B * C], f32)
            junk_a = sb.tile([P, FREE], f32)
            # 8 DMA groups of 2 planes; alternate DVE reduce / ACT accum
            for g in range(8):
                t_sb = tin.tile([P, 2, FREE], f8, tag="t")
                nc.sync.dma_start(t_sb[:], t[:, 2 * g : 2 * g + 2, :])
                if g % 2 == 0:
                    nc.vector.tensor_reduce(
                        out=cnt[:, 2 * g : 2 * g + 2], in_=t_sb[:],
                        axis=mybir.AxisListType.X, op=AL.add)
                else:
                    for j in range(2):
                        nc.scalar.activation(
                            out=junk_a[:], in_=t_sb[:, j, :], func=FA.Copy,
                            accum_out=cnt[:, 2 * g + j : 2 * g + j + 1])
            nc.sync.dma_start(out[:], cnt[:])
    nc.compile()
    return nc


def _tree(nc, s4, s2, planes, out_ap=None):
    """Pairwise-add a list of [P, F] APs using slices of the scratch tiles
    s4 ([P,4,F]) / s2 ([P,2,F]) for intermediates. The final add writes
    out_ap if given. Returns the final AP. len(planes) in [2, 8]."""
    from concourse import mybir
    AL = mybir.AluOpType
    scratch = [s4, s2, None]
    cur = list(planes)
    li = 0
    while len(cur) > 1:
        nxt = []
        k = 0
        for i in range(0, len(cur) - 1, 2):
            final = len(cur) == 2
            if final and out_ap is not None:
                dst = out_ap
            elif final and out_ap is None:
                # pick a slot that is never an input at this level
                dst = (s2 if li < 2 else s4)[:, 0, :]
            else:
                dst = scratch[li][:, k, :]
                k += 1
            nc.vector.tensor_tensor(out=dst, in0=cur[i], in1=cur[i + 1],
                                    op=AL.add)
            nxt.append(dst)
        if len(cur) % 2:
            nxt.append(cur[-1])
        cur = nxt
        li += 1
    return cur[0]


def _build_b(pattern):
    """pattern: tuple per sample of present-channel tuples."""
    import concourse.bacc as bacc
    import concourse.tile as tile
    from concourse import mybir

    FA = mybir.ActivationFunctionType
    AL = mybir.AluOpType
    f32, bf16 = mybir.dt.float32, mybir.dt.bfloat16

    pres = [list(p) for p in pattern]
    n = [len(p) for p in pres]
    L = max(n)
    pad = [float(L - nn) for nn in n]
    absent = [[c for c in range(C) if c not in p] for p in pres]

    nslots = sum(len(p) for p in pres)
    nc = bacc.Bacc("TRN2", num_devices=NCORES, name="loss_main_v2")
    # partition-major layouts; t holds only the present planes, packed
    # per sample in pres order
    x = nc.dram_tensor("x", [P, B * C, FREE], bf16, kind="ExternalInput")
    t = nc.dram_tensor("t", [P, nslots, FREE], bf16, kind="ExternalInput")
    out = nc.dram_tensor("out", [P, NACC], f32, kind="ExternalOutput")

    with tile.TileContext(nc) as tc:
        with (
            tc.tile_pool(name="persist", bufs=1) as persist,
            tc.tile_pool(name="ework", bufs=2) as ework,
            tc.tile_pool(name="small", bufs=2) as small,
            tc.psum_pool(name="ps", bufs=2) as psp,
        ):
            x_sb = persist.tile([P, B * C, FREE], bf16)
            # only present t planes are ever read — pack them
            tslot = {}
            for b in range(B):
                for c in pres[b]:
                    tslot[(b, c)] = len(tslot)
            t_sb = persist.tile([P, len(tslot), FREE], bf16)
            accs = persist.tile([P, NACC], f32)
            s4 = persist.tile([P, 4, FREE], bf16)
            s2 = persist.tile([P, 2, FREE], bf16)
            junk = persist.tile([P, FREE], bf16)
            nc.vector.memset(accs[:], 0.0)
            # sliding-window ones for per-channel matmul reductions:
            # W_c = wsl[:, C-1-c : 2*C-1-c] is [128, C] with ones in col c
            wsl = persist.tile([P, 2 * C - 1], bf16)
            nc.gpsimd.memset(wsl[:], 0.0)
            nc.gpsimd.memset(wsl[:, C - 1 : C], 1.0)
            CH = [(0, 400), (400, 800), (800, 1200), (1200, 1600)]

            def class_sums(planes, acc_col):
                """acc_col[c] (partition c) = sum over plane c, via PE."""
                ps = psp.tile([C, 400], f32, tag="ps")
                items = [(c, j) for c in range(len(planes))
                         for j in range(len(CH))]
                for idx, (c, j) in enumerate(items):
                    lo, hi = CH[j]
                    nc.tensor.matmul(
                        ps[:, 0 : hi - lo],
                        wsl[:, C - 1 - c : 2 * C - 1 - c],
                        planes[c][:, lo:hi],
                        start=(idx == 0), stop=(idx == len(items) - 1))
                nc.vector.tensor_reduce(
                    out=acc_col, in_=ps[:],
                    axis=mybir.AxisListType.X, op=AL.add)

            # DMA: few large loads (contiguous bc runs), ordered so each
            # sample's absent x planes (bg tree inputs) arrive first, and
            # t planes arrive between samples' x loads.
            def runs_of(ixs):
                rr = []
                for i in sorted(ixs):
                    if rr and rr[-1][1] == i:
                        rr[-1][1] = i + 1
                    else:
                        rr.append([i, i + 1])
                return rr

            x_groups, t_groups = [], []
            for b in range(B):
                gx = (runs_of([b * C + c for c in absent[b]]) +
                      runs_of([b * C + c for c in pres[b]]))
                gt = runs_of([tslot[(b, c)] for c in pres[b]])
                x_groups.append(gx)
                t_groups.append(gt)
            # issue order: x(b0), t(b0), x(b1), t(b1)
            for b in range(B):
                for lo, hi in x_groups[b]:
                    nc.sync.dma_start(x_sb[:, lo:hi, :], x[:, lo:hi, :])
                for lo, hi in t_groups[b]:
                    nc.sync.dma_start(t_sb[:, lo:hi, :], t[:, lo:hi, :])

            for b in range(B):
                xb = x_sb[:, b * C : (b + 1) * C, :]
                tb = lambda c: t_sb[:, tslot[(b, c)], :]
                # ---- bg merge: x0 += sum(absent x) (weights exactly 1)
                if len(absent[b]) == 1:
                    nc.vector.tensor_tensor(
                        out=xb[:, 0, :], in0=xb[:, 0, :],
                        in1=xb[:, absent[b][0], :], op=AL.add)
                elif absent[b]:
                    bg = _tree(nc, s4, s2, [xb[:, c, :] for c in absent[b]])
                    nc.vector.tensor_tensor(
                        out=xb[:, 0, :], in0=xb[:, 0, :], in1=bg, op=AL.add)

                # ---- e = exp(x) over present channels (contiguous runs)
                e = ework.tile([P, C, FREE], bf16, tag="e")
                runs = []
                for c in pres[b]:
                    if runs and runs[-1][1] == c:
                        runs[-1][1] = c + 1
                    else:
                        runs.append([c, c + 1])
                for lo, hi in runs:
                    nc.scalar.activation(
                        out=e[:, lo:hi, :], in_=xb[:, lo:hi, :], func=FA.Exp)

                # ---- S = sum_present e (f32); r = 1/S; rb = bf16 r
                S = small.tile([P, FREE], f32, tag="S")
                _tree(nc, s4, s2, [e[:, c, :] for c in pres[b]], out_ap=S[:])
                r = small.tile([P, FREE], f32, tag="r")
                nc.vector.reciprocal_approx_fast(r[:], S[:])
                rb = small.tile([P, FREE], bf16, tag="rb")
                nc.vector.tensor_scalar(rb[:], r[:], 1.0, None, AL.mult)

                # ---- q_c = e_c * r, overwriting x planes;
                #      tq_c = t_c * q_c, overwriting t planes;
                #      seg/intersect reduced per class on the PE
                for c in pres[b]:
                    nc.vector.tensor_tensor(
                        out=xb[:, c, :], in0=e[:, c, :], in1=rb[:],
                        op=AL.mult)
                class_sums([xb[:, c, :] for c in pres[b]],
                           accs[0:C, SEGB + b : SEGB + b + 1])
                for c in pres[b]:
                    nc.vector.tensor_tensor(
                        out=tb(c), in0=tb(c), in1=xb[:, c, :],
                        op=AL.mult)
                class_sums([tb(c) for c in pres[b]],
                           accs[0:C, INTB + b : INTB + b + 1])

                # ---- g_q = sum_c tq_c = q_label (exact one-hot select)
                gq = small.tile([P, FREE], bf16, tag="gq")
                _tree(nc, s4, s2, [tb(c) for c in pres[b]],
                      out_ap=gq[:])

                # ---- CE terms: u = sum ln(g_q) + sum ln(S); lse = ln(S+pad)
                nc.scalar.activation(
                    out=junk[:], in_=gq[:], func=FA.Ln,
                    accum_out=accs[:, U1 + b : U1 + b + 1])
                if pad[b] > 0:
                    padb = small.tile([P, 1], f32, tag="pad")
                    nc.vector.memset(padb[:], pad[b])
                    nc.scalar.activation(
                        out=junk[:], in_=S[:], func=FA.Ln, bias=padb[:],
                        accum_out=accs[:, LSE + b : LSE + b + 1])
                    nc.scalar.activation(
                        out=junk[:], in_=S[:], func=FA.Ln,
                        accum_out=accs[:, U2 + b : U2 + b + 1])
                else:
                    nc.scalar.activation(
                        out=junk[:], in_=S[:], func=FA.Ln,
                        accum_out=accs[:, LSE + b : LSE + b + 1])

            nc.sync.dma_start(out[:], accs[:])
    nc.compile()
    return nc


def _get(name, builder, *args):
    if name not in _CACHE:
        _CACHE[name] = builder(*args)
    return _CACHE[name]


def _shard_inputs(net_output, target):
    # [B,C,K,P,F] -> per-core partition-major [P, B*C, F]
    xs = np.asarray(net_output).reshape(B, C, NCORES, P, FREE)
    ts = np.asarray(target).reshape(B, C, NCORES, P, FREE)
    xpm = np.ascontiguousarray(
        xs.transpose(2, 3, 0, 1, 4).reshape(NCORES, P, B * C, FREE))
    tpm = np.ascontiguousarray(
        ts.transpose(2, 3, 0, 1, 4).reshape(NCORES, P, B * C, FREE))
    xmaps = [xpm[k].astype(ml_dtypes.bfloat16) for k in range(NCORES)]
    tpm16 = tpm.astype(ml_dtypes.bfloat16)           # one-hot: exact
    t8maps = [tpm[k].astype(ml_dtypes.float8_e4m3) for k in range(NCORES)]
    return xmaps, tpm16, t8maps


def _run(nc, in_maps, out_name):
    import os
    if os.environ.get("K_SIM", "0") == "1":
        import concourse.bass_interp as bass_interp
        sim = bass_interp.MultiCoreSim(nc, NCORES)
        for k in range(NCORES):
            for name, arr in in_maps[k].items():
                sim.cores[k].tensor(name)[:] = arr
        sim.simulate()
        return [{out_name: sim.cores[k].tensor(out_name).copy()}
                for k in range(NCORES)]
    from concourse.bass_utils import run_bass_kernel_spmd
    return run_bass_kernel_spmd(
        nc, in_maps, core_ids=list(range(NCORES))).results


def run_a(t8maps):
    nc = _get("a", _build_a)
    results = _run(nc, [{"t": tk} for tk in t8maps], "cnt")
    cnt_g = np.zeros((B, C), dtype=np.float64)
    for r in results:
        cnt_g += r["cnt"].astype(np.float64).sum(axis=0).reshape(B, C)
    return cnt_g


def run_b(xmaps, tpm16, pattern):
    nc = _get(("b", pattern), _build_b, pattern)
    pres_flat = [b * C + c for b in range(B) for c in pattern[b]]
    tmaps = [np.ascontiguousarray(tpm16[k][:, pres_flat, :])
             for k in range(NCORES)]
    in_maps = [{"x": xmaps[k], "t": tmaps[k]} for k in range(NCORES)]
    results = _run(nc, in_maps, "out")
    acc = np.zeros((P, NACC), dtype=np.float64)
    for r in results:
        acc += r["out"].astype(np.float64)
    return acc


def _finish(cnt_g, acc, present, n):
    pad = n.max() - n
    # SEGB/INTB: per-class values live at partition = position in the
    # present-channel list of that sample
    seg = np.zeros((B, C)); inter = np.zeros((B, C))
    for b in range(B):
        pres = np.where(present[b])[0]
        seg[b, pres] = acc[: len(pres), SEGB + b]
        inter[b, pres] = acc[: len(pres), INTB + b]
    cols = acc.sum(axis=0)
    u1 = cols[U1 : U1 + B]
    lse = cols[LSE : LSE + B]
    u2 = np.where(pad > 0, cols[U2 : U2 + B], lse)
    ce = (lse.sum() - (u1.sum() + u2.sum())) / NVOX
    dice_c = 2.0 * inter / (cnt_g + seg + 1e-5)
    dice_i = 1.0 - (present * dice_c).sum(axis=1) / n
    dc = dice_i.mean()
    return np.asarray(0.5 * ce + 0.5 * dc, dtype=np.float32)


def kernel(net_output, target):
    xmaps, tpm16, t8maps = _shard_inputs(
        np.asarray(net_output), np.asarray(target))
    cnt_g = run_a(t8maps)
    present = cnt_g > 0.5
    n = present.sum(axis=1).astype(np.float64)
    pattern = tuple(tuple(int(c) for c in np.where(present[b])[0])
                    for b in range(B))
    acc = run_b(xmaps, tpm16, pattern)
    return _finish(cnt_g, acc, present, n)
